# revision 17
# baseline (speedup 1.0000x reference)
"""DNANet Bass kernel v4: 2x-DVE onehots, transposed AllGather, aug-bias.

Scheme (per core, edges sharded by dst range):
- kv table [npad, 3*(k64|v64)] bf16 in DRAM, rebuilt slice-by-slice per
  layer from replicated node features (bf16 matmuls, bias folded into an
  augmented 65-row contraction). v is pre-scaled by dinv[src]. q tables
  for the edge passes are built from the core's own layer outputs and
  kept SBUF resident.
- Layer outputs AllGather in TRANSPOSED form ([HID, nsh] per core), so
  table builds consume them directly as matmul lhsT without per-block
  PE transposes.
- Edge pass per dst tile (128 nodes): superchunks of <=8 chunks of 128
  edges. Per superchunk: SWDGE dma_gather of kv rows (4 queues,
  -1-padded indices), onehot oh / transposed onehot ohT via is_equal
  against host-replicated bf16 tables (all-bf16 packed operands -> 2x
  DVE), per-edge q via ohT matmuls, fused qk mult/reduce (bf16),
  softmax over layer history, ACT-expanded attention weights feeding a
  2x message mult, scatter via oh matmuls into PSUM.
"""
import math
import numpy as np
import concourse.bacc as bacc
import concourse.mybir as mybir
import concourse.tile as tile
from concourse.masks import make_identity

F32, I32, I16 = mybir.dt.float32, mybir.dt.int32, mybir.dt.int16
BF16 = mybir.dt.bfloat16
HID, H, DH, INC, OUTC = 64, 4, 16, 128, 16
LMAX = 3
KVROW = LMAX * 2 * HID  # 384 elements per node row in kv table
HALF = 25088            # row split so int16 indices stay positive
C_MAX = 8               # chunks per superchunk (<=1024 idx per dma_gather)
NQ = 4                  # SWDGE queues


def _wrap16(arr_cm):
    """[tot_ch, 128] int -> [128, tot_ch*8] int16 ucode idx layout.

    Replicated into all eight 16-partition groups so any SWDGE queue's
    Q7 core pair can read its copy."""
    tc_, _ = arr_cm.shape
    a = arr_cm.reshape(tc_, 8, 16).transpose(2, 0, 1).reshape(16, tc_ * 8)
    out = np.zeros((128, tc_ * 8), np.int16)
    for r in range(8):
        out[r * 16:(r + 1) * 16] = a
    return out


def make_cfg(n_nodes, edge_index, n_cores=8):
    tpc = math.ceil(n_nodes / n_cores / 128)
    npad = n_cores * tpc * 128
    nsh = tpc * 128
    half = HALF if npad + 128 > 32000 else npad + 128

    src = np.asarray(edge_index[0], dtype=np.int64)
    dst = np.asarray(edge_index[1], dtype=np.int64)
    loop = np.arange(n_nodes, dtype=np.int64)
    src = np.concatenate([src, loop])
    dst = np.concatenate([dst, loop])

    deg = np.bincount(dst, minlength=npad).astype(np.float32)
    dinv = np.where(deg > 0, 1.0 / np.sqrt(np.maximum(deg, 1.0)), 0.0).astype(np.float32)

    key = dst * 2 + (src >= half)
    order = np.argsort(key, kind="stable")
    src_s, dst_s = src[order], dst[order]

    cnt_lo = np.zeros((n_cores, tpc), np.int64)
    cnt_hi = np.zeros((n_cores, tpc), np.int64)
    core_of = dst_s // nsh
    ltile = (dst_s % nsh) // 128
    is_hi = src_s >= half
    np.add.at(cnt_lo, (core_of[~is_hi], ltile[~is_hi]), 1)
    np.add.at(cnt_hi, (core_of[is_hi], ltile[is_hi]), 1)
    kt_lo = np.maximum(np.ceil(cnt_lo.max(axis=0) / 128).astype(np.int64), 1)
    kt_hi = np.ceil(cnt_hi.max(axis=0) / 128).astype(np.int64)
    kt = kt_lo + kt_hi
    tot_ch = int(kt.sum())
    chunk_base = np.concatenate([[0], np.cumsum(kt)]).astype(np.int64)

    kvidx_cm = np.zeros((n_cores, tot_ch, 128), np.int64)
    dloc_cm = np.full((n_cores, tot_ch, 128), 128, np.int64)
    for c in range(n_cores):
        for t in range(tpc):
            g0 = c * nsh + t * 128
            lo = np.searchsorted(dst_s, g0)
            hi = np.searchsorted(dst_s, g0 + 128)
            if hi == lo:
                continue
            sseg = src_s[lo:hi]
            dseg = dst_s[lo:hi]
            hseg = sseg >= half
            for half_i, mask, base_ch in (
                (0, ~hseg, chunk_base[t]),
                (1, hseg, chunk_base[t] + kt_lo[t]),
            ):
                sv = sseg[mask]
                dv = dseg[mask]
                n = len(sv)
                if n == 0:
                    continue
                ch = base_ch + np.arange(n) // 128
                lane = np.arange(n) % 128
                kvidx_cm[c, ch, lane] = sv - (half if half_i else 0)
                dloc_cm[c, ch, lane] = dv - g0

    # superchunk list: (chunk offset, n chunks, row base, sc index)
    scs_per_tile = []
    nsc = 0
    for t in range(tpc):
        scs = []
        for seg_o, seg_n, rb in ((0, int(kt_lo[t]), 0),
                                 (int(kt_lo[t]), int(kt_hi[t]), half)):
            o = 0
            while o < seg_n:
                w = min(C_MAX, seg_n - o)
                scs.append((seg_o + o, w, rb, nsc))
                o += w
                nsc += 1
        scs_per_tile.append(scs)
    assert nsc <= 128, f"superchunk count {nsc} exceeds 128 partitions"

    # dlocrow[c, sc, e]: dloc of edge e of superchunk sc (sentinel 128)
    dlocrow = np.full((n_cores, max(nsc, 1), C_MAX * 128), 128.0, np.float32)
    for t in range(tpc):
        for (o, cc, rb, sc) in scs_per_tile[t]:
            cb = int(chunk_base[t]) + o
            for c in range(n_cores):
                dlocrow[c, sc, :cc * 128] = (
                    dloc_cm[c, cb:cb + cc].reshape(-1).astype(np.float32))

    dinvT = dinv.reshape(-1, 128).T.copy()  # [128, nt_g]
    return dict(
        n_cores=n_cores, tpc=tpc, npad=npad, nsh=nsh, n_nodes=n_nodes,
        half=half,
        kt=[int(k) for k in kt], kt_lo=[int(k) for k in kt_lo],
        kt_hi=[int(k) for k in kt_hi],
        tot_ch=tot_ch, chunk_base=[int(b) for b in chunk_base],
        scs_per_tile=scs_per_tile, nsc=nsc,
        kvidx=np.stack([_wrap16(kvidx_cm[c].astype(np.int16)) for c in range(n_cores)]),
        dloc_cm=dloc_cm,
        dlocrow=dlocrow,
        dinvT=dinvT,
    )


def prep_inputs(cfg, x, W_lin, b_lin, Wq, bq, Wk, bk, Wv, bv, W_out, b_out):
    import ml_dtypes
    ncore, npad, tpc = cfg["n_cores"], cfg["npad"], cfg["tpc"]
    nsc, tot_ch = cfg["nsc"], cfg["tot_ch"]
    n = x.shape[0]
    bf = lambda a: np.ascontiguousarray(
        np.asarray(a, np.float32).astype(ml_dtypes.bfloat16))
    xT = np.zeros((INC, npad), ml_dtypes.bfloat16)
    xT[:, :n] = np.asarray(x, np.float32).T.astype(ml_dtypes.bfloat16)

    def aug(W, b):  # [65, out]: weights with bias folded as extra row
        return bf(np.concatenate(
            [np.asarray(W, np.float32),
             np.asarray(b, np.float32).reshape(1, -1)], axis=0))

    pidrep = np.repeat(np.arange(128, dtype=np.float32)[:, None], C_MAX * 128, 1)
    base = {
        "xT": xT,
        "wlin": bf(W_lin),
        "wq_a": aug(Wq, bq),
        "wk_a": aug(Wk, bk),
        "wv_a": aug(Wv, bv),
        "wout_a": aug(W_out, b_out),
        "blin_col": np.asarray(b_lin, np.float32).reshape(HID, 1),
        "iota_bf": bf(np.broadcast_to(np.arange(128, dtype=np.float32),
                                      (128, 128))),
        "pidrep": bf(pidrep),
        "dinvT": cfg["dinvT"],
    }
    in_maps = []
    for c in range(ncore):
        m = dict(base)
        m["dinvL"] = np.ascontiguousarray(cfg["dinvT"][:, c * tpc:(c + 1) * tpc])
        m["kvidx"] = cfg["kvidx"][c]
        # ohT table: dloc replicated across dst partitions, per superchunk
        flat = cfg["dlocrow"][c].reshape(1, nsc * C_MAX * 128)
        m["dlocbc"] = bf(np.broadcast_to(flat, (128, nsc * C_MAX * 128)))
        # oh table: per edge-lane dloc replicated across the 128 iota slots
        dl = cfg["dloc_cm"][c].astype(np.float32)  # [tot_ch, 128] (ch, lane)
        dlibc = np.repeat(dl.transpose(1, 0).reshape(128, tot_ch, 1), 128,
                          axis=2).reshape(128, tot_ch * 128)
        m["dlibc"] = bf(dlibc)
        in_maps.append(m)
    return in_maps


def build_kernel(cfg):
    ncore, tpc, npad, nsh = cfg["n_cores"], cfg["tpc"], cfg["npad"], cfg["nsh"]
    kt, kt_lo, kt_hi = cfg["kt"], cfg["kt_lo"], cfg["kt_hi"]
    tot_ch, chunk_base, half = cfg["tot_ch"], cfg["chunk_base"], cfg["half"]
    scs_per_tile = cfg["scs_per_tile"]
    nt_g = ncore * tpc
    ng = nt_g // 4

    nc = bacc.Bacc("TRN2", target_bir_lowering=False, debug=False,
                   num_devices=ncore, num_swdge_queues=NQ)

    xT = nc.dram_tensor("xT", [INC, npad], BF16, kind="ExternalInput")
    wlin = nc.dram_tensor("wlin", [INC, HID], BF16, kind="ExternalInput")
    wq_a = nc.dram_tensor("wq_a", [HID + 1, HID], BF16, kind="ExternalInput")
    wk_a = nc.dram_tensor("wk_a", [HID + 1, HID], BF16, kind="ExternalInput")
    wv_a = nc.dram_tensor("wv_a", [HID + 1, HID], BF16, kind="ExternalInput")
    wout_a = nc.dram_tensor("wout_a", [HID + 1, OUTC], BF16, kind="ExternalInput")
    blin_col = nc.dram_tensor("blin_col", [HID, 1], F32, kind="ExternalInput")
    iota_bf_d = nc.dram_tensor("iota_bf", [128, 128], BF16, kind="ExternalInput")
    pidrep_d = nc.dram_tensor("pidrep", [128, C_MAX * 128], BF16,
                              kind="ExternalInput")
    dinvT_d = nc.dram_tensor("dinvT", [128, nt_g], F32, kind="ExternalInput")
    dinvL_d = nc.dram_tensor("dinvL", [128, tpc], F32, kind="ExternalInput")
    kvidx_d = nc.dram_tensor("kvidx", [128, tot_ch * 8], I16, kind="ExternalInput")
    dlocbc_d = nc.dram_tensor("dlocbc", [128, cfg["nsc"] * C_MAX * 128], BF16,
                              kind="ExternalInput")
    dlibc_d = nc.dram_tensor("dlibc", [128, tot_ch * 128], BF16,
                             kind="ExternalInput")
    y = nc.dram_tensor("y", [nsh, OUTC], F32, kind="ExternalOutput")

    with tile.TileContext(nc) as tc:
        import contextlib
        ctx = contextlib.ExitStack()
        with ctx:
            cpool = ctx.enter_context(tc.tile_pool(name="const", bufs=1))
            dram = ctx.enter_context(tc.tile_pool(name="dram", bufs=1, space="DRAM"))

            kvtab = dram.tile([npad + 128, KVROW], BF16, name="kvtab")
            aginT = [dram.tile([HID, nsh], BF16, name=f"aginT{s}") for s in (1, 2)]
            agoutT = [dram.tile([ncore * HID, nsh], BF16, name=f"agoutT{s}",
                                addr_space="Shared") for s in (1, 2)]

            def load_const(dt_, shape, src_ap, name):
                t_ = cpool.tile(shape, dt_, name=name)
                nc.sync.dma_start(t_[:], src_ap)
                return t_

            wlin_s = load_const(BF16, [INC, HID], wlin[:], "wlin_s")
            wq_s = load_const(BF16, [HID + 1, HID], wq_a[:], "wq_s")
            wk_s = load_const(BF16, [HID + 1, HID], wk_a[:], "wk_s")
            wv_s = load_const(BF16, [HID + 1, HID], wv_a[:], "wv_s")
            wout_s = load_const(BF16, [HID + 1, OUTC], wout_a[:], "wout_s")
            blin_s = load_const(F32, [HID, 1], blin_col[:], "blin_s")
            iota_s = load_const(BF16, [128, 128], iota_bf_d[:], "iota_s")
            pidrep_s = load_const(BF16, [128, C_MAX * 128], pidrep_d[:], "pidrep_s")
            dinvT_s = load_const(F32, [128, nt_g], dinvT_d[:], "dinvT_s")
            dinvL_s = load_const(F32, [128, tpc], dinvL_d[:], "dinvL_s")
            kvidx_s = load_const(I16, [128, tot_ch * 8], kvidx_d[:], "kvidx_s")
            iden = cpool.tile([128, 128], F32, name="iden")
            make_identity(nc, iden[:])
            iden_bf = cpool.tile([128, 128], BF16, name="iden_bf")
            nc.vector.tensor_copy(iden_bf[:], iden[:])
            qtabs = [cpool.tile([128, tpc * HID], BF16, name=f"qtab{i}")
                     for i in (0, 1)]

            sb_xt = ctx.enter_context(tc.tile_pool(name="sb_xt", bufs=3))
            sb_ht = ctx.enter_context(tc.tile_pool(name="sb_ht", bufs=3))
            sb_kv = ctx.enter_context(tc.tile_pool(name="sb_kv", bufs=3))
            sb_oh = ctx.enter_context(tc.tile_pool(name="sb_oh", bufs=4))
            sb_oht = ctx.enter_context(tc.tile_pool(name="sb_oht", bufs=4))
            sb_g = ctx.enter_context(tc.tile_pool(name="sb_g", bufs=8))
            sb_qe = ctx.enter_context(tc.tile_pool(name="sb_qe", bufs=3))
            sb_ve = ctx.enter_context(tc.tile_pool(name="sb_ve", bufs=3))
            sb_sm = ctx.enter_context(tc.tile_pool(name="sb_sm", bufs=4))
            sb_out = ctx.enter_context(tc.tile_pool(name="sb_out", bufs=3))
            ps_big = ctx.enter_context(tc.tile_pool(name="ps_big", bufs=1, space="PSUM"))
            ps_kv = ctx.enter_context(tc.tile_pool(name="ps_kv", bufs=1, space="PSUM"))
            ps_out = ctx.enter_context(tc.tile_pool(name="ps_out", bufs=2, space="PSUM"))
            ps_qe = ctx.enter_context(tc.tile_pool(name="ps_qe", bufs=1, space="PSUM"))

            AF, ALU = mybir.ActivationFunctionType, mybir.AluOpType
            AX = mybir.AxisListType
            qctr = [0]

            def next_q():
                qctr[0] += 1
                return qctr[0] % NQ

            # zero the gather buffers once (pad lanes are never written and
            # must stay finite); set the ones row of the hts buffers once
            for _ in range(8):
                tmp = sb_g.tile([128, C_MAX * LMAX * 128], BF16, name="kvg")
                nc.vector.memset(tmp[:], 0.0)
                tmp = sb_g.tile([128, C_MAX * 128], BF16, name="vg1")
                nc.vector.memset(tmp[:], 0.0)
            for _ in range(3):
                tmp = sb_ht.tile([HID + 1, 512], BF16, name="hts")
                nc.vector.memset(tmp[HID:HID + 1, :], 1.0)

            # ================= table slice build =================
            def build_slice(s):
                for g in range(ng):
                    hts = sb_ht.tile([HID + 1, 512], BF16, name="hts")
                    if s == 0:
                        xt_t = sb_xt.tile([INC, 512], BF16, name="xt_t")
                        nc.sync.dma_start(xt_t[:], xT[:, g * 512:(g + 1) * 512])
                        htp = ps_big.tile([HID, 512], F32, name="htp", space="PSUM")
                        nc.tensor.matmul(htp[:], lhsT=wlin_s[:], rhs=xt_t[:],
                                         start=True, stop=True)
                        nc.scalar.activation(hts[:HID, :], htp[:], AF.Identity,
                                             bias=blin_s[:], scale=1.0)
                    else:
                        # hts rows 0..63 straight from the transposed AllGather
                        g0 = g * 512
                        while g0 < (g + 1) * 512:
                            c = g0 // nsh
                            w = min((g + 1) * 512 - g0, (c + 1) * nsh - g0)
                            nc.sync.dma_start(
                                hts[:HID, g0 - g * 512:g0 - g * 512 + w],
                                agoutT[s - 1][c * HID:(c + 1) * HID,
                                              g0 - c * nsh:g0 - c * nsh + w])
                            g0 += w

                    kp = ps_kv.tile([128, 4 * HID], F32, name="kp", space="PSUM")
                    vp = ps_kv.tile([128, 4 * HID], F32, name="vp", space="PSUM")
                    for b in range(4):
                        nc.tensor.matmul(kp[:, b * HID:(b + 1) * HID],
                                         lhsT=hts[:, b * 128:(b + 1) * 128],
                                         rhs=wk_s[:],
                                         start=(b == 0), stop=(b == 3))
                    for b in range(4):
                        nc.tensor.matmul(vp[:, b * HID:(b + 1) * HID],
                                         lhsT=hts[:, b * 128:(b + 1) * 128],
                                         rhs=wv_s[:],
                                         start=(b == 0), stop=(b == 3))

                    kvsb = sb_kv.tile([128, 4 * 128], BF16, name="kvsb")
                    kvv = kvsb[:].rearrange("p (b s d) -> p b s d", b=4, s=2, d=HID)
                    nc.scalar.activation(
                        kvv[:, :, 0, :],
                        kp[:].rearrange("p (b d) -> p b d", b=4, d=HID),
                        AF.Copy)
                    nc.vector.tensor_tensor(
                        kvv[:, :, 1, :],
                        vp[:].rearrange("p (b d) -> p b d", b=4, d=HID),
                        dinvT_s[:, g * 4:(g + 1) * 4]
                        .rearrange("p (b u) -> p b u", b=4, u=1)
                        .to_broadcast([128, 4, HID]),
                        ALU.mult)
                    nc.sync.dma_start(
                        kvtab[g * 512:(g + 1) * 512, s * 128:(s + 1) * 128]
                        .rearrange("(b p) d -> p b d", p=128),
                        kvsb[:].rearrange("p (b d) -> p b d", b=4, d=128))

            # q table for this core's nodes from its transposed layer output
            def build_qtab(qi, agidx):
                ngl = math.ceil(tpc / 4)
                for gl in range(ngl):
                    wb = min(4, tpc - gl * 4)
                    hts = sb_ht.tile([HID + 1, 512], BF16, name="hts")
                    nc.sync.dma_start(
                        hts[:HID, :wb * 128],
                        aginT[agidx][:, gl * 512:gl * 512 + wb * 128])
                    qp = ps_kv.tile([128, 4 * HID], F32, name="kp", space="PSUM")
                    for b in range(wb):
                        nc.tensor.matmul(qp[:, b * HID:(b + 1) * HID],
                                         lhsT=hts[:, b * 128:(b + 1) * 128],
                                         rhs=wq_s[:],
                                         start=(b == 0), stop=(b == wb - 1))
                    nc.scalar.activation(
                        qtabs[qi][:, gl * 4 * HID:(gl * 4 + wb) * HID],
                        qp[:, :wb * HID], AF.Copy)

            # ================= edge pass =================
            def edge_layer(ell, agidx):
                L = ell
                for t in range(tpc):
                    po = ps_out.tile([128, HID], F32, name="po", space="PSUM")
                    qtile = qtabs[ell - 2][:, t * HID:(t + 1) * HID] if ell > 1 else None
                    n_ch = kt[t]
                    base = chunk_base[t]
                    done = 0
                    for (o, cc, rb, sc) in scs_per_tile[t]:
                        cb = base + o
                        ne = cc * 128

                        # oh[e_p, c, i] via host-replicated dloc (2x is_equal)
                        oh = sb_oh.tile([128, C_MAX * 128], BF16, name="oh")
                        dli = sb_oh.tile([128, C_MAX * 128], BF16, name="dli")
                        nc.sync.dma_start(
                            dli[:, :ne], dlibc_d[:, cb * 128:cb * 128 + ne])
                        nc.vector.tensor_tensor(
                            oh[:, :ne].rearrange("p (c i) -> p c i", c=cc, i=128),
                            dli[:, :ne].rearrange("p (c i) -> p c i", c=cc, i=128),
                            iota_s[:].rearrange("p (u i) -> p u i", u=1, i=128)
                            .to_broadcast([128, cc, 128]),
                            ALU.is_equal)

                        if ell == 1:
                            vg = sb_g.tile([128, C_MAX * 128], BF16, name="vg1")
                            nc.gpsimd.dma_gather(
                                out_ap=vg[:, :ne].rearrange(
                                    "p (n d) -> p n d", d=128),
                                in_ap=kvtab[rb:, 0:128],
                                idxs_ap=kvidx_s[:, cb * 8:(cb + cc) * 8],
                                num_idxs=ne, num_idxs_reg=ne,
                                elem_size=128, elem_step=KVROW,
                                queue_num=next_q())
                            vgv = vg[:, :ne].rearrange("p (c s d) -> p c s d",
                                                       s=2, d=HID)
                            mslice = lambda k: vgv[:, k, 1, :]
                        else:
                            kvg = sb_g.tile([128, C_MAX * LMAX * 128], BF16, name="kvg")
                            nc.gpsimd.dma_gather(
                                out_ap=kvg[:, :cc * L * 128].rearrange(
                                    "p (n d) -> p n d", d=L * 128),
                                in_ap=kvtab[rb:, :L * 128],
                                idxs_ap=kvidx_s[:, cb * 8:(cb + cc) * 8],
                                num_idxs=ne, num_idxs_reg=ne,
                                elem_size=L * 128, elem_step=KVROW,
                                queue_num=next_q())

                            # ohT[d_p, c, e] via host-replicated dloc rows
                            dls = sb_oht.tile([128, C_MAX * 128], BF16,
                                              name="dls")
                            nc.sync.dma_start(
                                dls[:, :ne],
                                dlocbc_d[:, sc * C_MAX * 128:
                                         sc * C_MAX * 128 + ne])
                            ohT = sb_oht.tile([128, C_MAX * 128], BF16, name="ohT")
                            nc.vector.tensor_tensor(
                                ohT[:, :ne], dls[:, :ne], pidrep_s[:, :ne],
                                ALU.is_equal)

                            # per-edge q via ohT chunks
                            qe = ps_qe.tile([128, C_MAX * HID], F32, name="qe",
                                            space="PSUM")
                            for k in range(cc):
                                nc.tensor.matmul(qe[:, k * HID:(k + 1) * HID],
                                                 lhsT=ohT[:, k * 128:(k + 1) * 128],
                                                 rhs=qtile,
                                                 start=True, stop=True)
                            qeb = sb_qe.tile([128, C_MAX * HID], BF16, name="qeb")
                            nc.scalar.activation(qeb[:, :cc * HID], qe[:, :cc * HID],
                                                 AF.Copy)

                            # fused qk: one mult + one reduce over d (bf16)
                            qkp = sb_ve.tile([128, C_MAX * LMAX * HID], BF16,
                                             name="qkp")
                            nc.vector.tensor_tensor(
                                qkp[:, :cc * L * HID].rearrange(
                                    "p (c l e) -> p c l e", l=L, e=HID),
                                kvg[:, :cc * L * 128].rearrange(
                                    "p (c l s e) -> p c l s e", l=L, s=2, e=HID)
                                [:, :, :, 0, :],
                                qeb[:, :cc * HID]
                                .rearrange("p (c u e) -> p c u e", u=1, e=HID)
                                .to_broadcast([128, cc, L, HID]),
                                ALU.mult)
                            # scores, memory layout [c, l, h], bf16
                            scr = sb_sm.tile([128, C_MAX * LMAX * H], BF16,
                                             name="scr")
                            with nc.allow_low_precision(reason="bf16 scores"):
                                nc.vector.reduce_sum(
                                    scr[:, :cc * L * H],
                                    qkp[:, :cc * L * HID].rearrange(
                                        "p (m d) -> p m d", d=DH),
                                    axis=AX.X)
                            eatt = sb_sm.tile([128, C_MAX * LMAX * H], BF16,
                                              name="eatt")
                            nc.scalar.activation(eatt[:, :cc * L * H],
                                                 scr[:, :cc * L * H],
                                                 AF.Exp, scale=1.0 / math.sqrt(DH))
                            e3 = eatt[:, :cc * L * H].rearrange(
                                "p (c l h) -> p c l h", l=L, h=H)
                            den = sb_sm.tile([128, C_MAX * H], F32, name="den")
                            d3 = den[:, :cc * H].rearrange("p (c h) -> p c h", h=H)
                            nc.vector.tensor_tensor(
                                d3, e3[:, :, 0, :], e3[:, :, 1, :], ALU.add)
                            if L > 2:
                                nc.vector.tensor_tensor(
                                    d3, d3, e3[:, :, 2, :], ALU.add)
                            rden = sb_sm.tile([128, C_MAX * H], BF16, name="rden")
                            with nc.allow_low_precision(reason="bf16 attn weights"):
                                nc.vector.reciprocal(rden[:, :cc * H], den[:, :cc * H])
                            att = sb_sm.tile([128, C_MAX * LMAX * H], BF16,
                                             name="att")
                            nc.vector.tensor_tensor(
                                att[:, :cc * L * H].rearrange(
                                    "p (c l h) -> p c l h", l=L, h=H),
                                e3,
                                rden[:, :cc * H]
                                .rearrange("p (c u h) -> p c u h", u=1, h=H)
                                .to_broadcast([128, cc, L, H]),
                                ALU.mult)
                            # expand attention weights over d on ACT so the
                            # message mult has all-packed operands (2x)
                            attx = sb_ve.tile([128, C_MAX * LMAX * HID], BF16,
                                              name="attx")
                            nc.scalar.activation(
                                attx[:, :cc * L * HID].rearrange(
                                    "p (m h d) -> p m h d", h=H, d=DH),
                                att[:, :cc * L * H]
                                .rearrange("p (m h u) -> p m h u", h=H, u=1)
                                .to_broadcast([128, cc * L, H, DH]),
                                AF.Copy)
                            # fused message: one 2x mult + (L-1) adds
                            wvt = sb_ve.tile([128, C_MAX * LMAX * HID], BF16,
                                             name="wvt")
                            nc.vector.tensor_tensor(
                                wvt[:, :cc * L * HID].rearrange(
                                    "p (m e) -> p m e", e=HID),
                                kvg[:, :cc * L * 128].rearrange(
                                    "p (m s e) -> p m s e", s=2, e=HID)
                                [:, :, 1, :],
                                attx[:, :cc * L * HID].rearrange(
                                    "p (m e) -> p m e", e=HID),
                                ALU.mult)
                            wvv = wvt[:, :cc * L * HID].rearrange(
                                "p (c l e) -> p c l e", l=L, e=HID)
                            msg = sb_ve.tile([128, C_MAX * HID], BF16, name="msg")
                            mv = msg[:, :cc * HID].rearrange(
                                "p (c e) -> p c e", e=HID)
                            nc.vector.tensor_tensor(
                                mv, wvv[:, :, 0, :], wvv[:, :, 1, :], ALU.add)
                            if L > 2:
                                nc.vector.tensor_tensor(
                                    mv, mv, wvv[:, :, 2, :], ALU.add)
                            mslice = lambda k: msg[:, k * HID:(k + 1) * HID]

                        for k in range(cc):
                            nc.tensor.matmul(po[:], lhsT=oh[:, k * 128:(k + 1) * 128],
                                             rhs=mslice(k),
                                             start=(done + k == 0),
                                             stop=(done + k == n_ch - 1))
                        done += cc

                    outsb = sb_out.tile([128, HID], F32, name="outsb")
                    nc.scalar.activation(outsb[:], po[:], AF.Copy,
                                         scale=dinvL_s[:, t:t + 1])
                    if ell < 3:
                        outbf = sb_out.tile([128, HID], BF16, name="outbf")
                        nc.vector.tensor_copy(outbf[:], outsb[:])
                        otp = ps_big.tile([HID, 128], BF16, name="otp",
                                          space="PSUM")
                        nc.tensor.transpose(otp[:], in_=outbf[:],
                                            identity=iden_bf[:])
                        otb = sb_out.tile([HID, 128], BF16, name="otb")
                        nc.scalar.activation(otb[:], otp[:], AF.Copy)
                        nc.sync.dma_start(
                            aginT[agidx][:, t * 128:(t + 1) * 128], otb[:])
                    else:
                        final_tile(outsb, t)

            def final_tile(outsb, t):
                outbf = sb_out.tile([128, HID], BF16, name="outbf")
                nc.vector.tensor_copy(outbf[:], outsb[:])
                otp = ps_big.tile([HID, 128], BF16, name="otp", space="PSUM")
                nc.tensor.transpose(otp[:], in_=outbf[:], identity=iden_bf[:])
                hts = sb_ht.tile([HID + 1, 512], BF16, name="hts")
                nc.scalar.activation(hts[:HID, :128], otp[:], AF.Copy)
                yp = ps_kv.tile([128, 4 * HID], F32, name="kp", space="PSUM")
                nc.tensor.matmul(yp[:, :OUTC], lhsT=hts[:, :128], rhs=wout_s[:],
                                 start=True, stop=True)
                ysb = sb_out.tile([128, OUTC], F32, name="ysb")
                nc.scalar.activation(ysb[:], yp[:, :OUTC], AF.Copy)
                nc.sync.dma_start(y[t * 128:(t + 1) * 128, :], ysb[:])

            # ================= schedule =================
            build_slice(0)
            edge_layer(1, 0)
            nc.gpsimd.collective_compute(
                "AllGather", mybir.AluOpType.bypass,
                replica_groups=[list(range(ncore))],
                ins=[aginT[0].opt()], outs=[agoutT[0].opt()])
            build_slice(1)
            build_qtab(0, 0)
            edge_layer(2, 1)
            nc.gpsimd.collective_compute(
                "AllGather", mybir.AluOpType.bypass,
                replica_groups=[list(range(ncore))],
                ins=[aginT[1].opt()], outs=[agoutT[1].opt()])
            build_slice(2)
            build_qtab(1, 1)
            edge_layer(3, None)

    nc.compile()
    return nc


def assemble_output(cfg, results):
    n = cfg["n_nodes"]
    full = np.concatenate([results[c]["y"] for c in range(cfg["n_cores"])], axis=0)
    return full[:n]


# ======================= harness entry point =======================
LAST_EXEC_NS = [None]
LAST_RES = [None]


def kernel(**inputs):
    """Full (unsharded) inputs -> full [N, 16] float32 output.

    Shards edges by destination range across the 8 NeuronCores, compiles
    the SPMD Bass kernel for this edge layout, runs it, and reassembles
    the per-core output shards.
    """
    from concourse.bass_utils import run_bass_kernel_spmd

    x = np.asarray(inputs["x"], np.float32)
    edge_index = np.asarray(inputs["edge_index"])
    cfg = make_cfg(x.shape[0], edge_index, n_cores=8)
    in_maps = prep_inputs(
        cfg, x,
        inputs["W_lin"], inputs["b_lin"],
        inputs["Wq"], inputs["bq"],
        inputs["Wk"], inputs["bk"],
        inputs["Wv"], inputs["bv"],
        inputs["W_out"], inputs["b_out"],
    )
    nc = build_kernel(cfg)
    res = run_bass_kernel_spmd(nc, in_maps, core_ids=list(range(cfg["n_cores"])))
    LAST_EXEC_NS[0] = res.exec_time_ns
    LAST_RES[0] = res
    return assemble_output(cfg, res.results)
